# revision 29
# baseline (speedup 1.0000x reference)
"""Causal self-attention block (QKV proj -> 16-head causal attention ->
out proj -> residual + LayerNorm) on 8 Trainium2 NeuronCores.

Sharding: the 4x2048 tokens are split into 16 blocks of 512; core i owns
blocks {i, 15-i}, which balances causal-attention work exactly (every
core attends to 2560 key-tokens total). Two SPMD launches:
  L1: QKV projection (uniform program, 8 cores). Emits q^T/k^T in
      [feature, token] layout and V in natural [token, feature] layout.
  L2: attention + out-proj + LayerNorm. Scores are computed transposed
      (S^T[tk, tq]) so softmax needs no partition reduction: exp runs on
      ScalarE (no max-subtraction needed -- scores are O(1) here), and
      the denominator falls out of a ones-column appended to V. Two
      program variants (block extents (512, 2048) and (1024, 1536))
      dispatched concurrently on disjoint 4-device submeshes.
Matmuls run in bf16 (1 cycle/row) except the QKV projection which uses
float32r (full fp32 at 1 cycle/row for N>=512).
"""

import os
import sys

for _p in ("/opt/trn_rl_repo",):
    if os.path.isdir(_p) and _p not in sys.path:
        sys.path.insert(0, _p)

import numpy as np
import ml_dtypes

import concourse.bass as bass
import concourse.mybir as mybir
import concourse.tile as tile
from concourse.vector_clock import ScopedClock

BF16 = ml_dtypes.bfloat16

B, S, E = 4, 2048, 1024
H, D = 16, 64
P = 128
BLK = 512  # token block
NBLK = 16  # global 512-token blocks
LN_EPS = 1e-5

# core i owns global blocks {i, 15-i}; process smaller causal extent first.
CORE_BLOCKS = []
for i in range(8):
    g1, g2 = i, 15 - i
    bl = sorted((g1, g2), key=lambda g: g % 4)
    CORE_BLOCKS.append(tuple(bl))


def _extents(core):
    return tuple((g % 4 + 1) * BLK for g in CORE_BLOCKS[core])


VARIANT_A = (512, 2048)  # cores 0,3,4,7
VARIANT_B = (1024, 1536)  # cores 1,2,5,6
CORES_OF_VARIANT = {
    VARIANT_A: [c for c in range(8) if _extents(c) == VARIANT_A],
    VARIANT_B: [c for c in range(8) if _extents(c) == VARIANT_B],
}


class _TileContext(tile.TileContext):
    """This walrus build rejects >1 sync-wait per instruction; move extra
    waits onto preceding same-engine NOPs (and extra chained drains for
    the Tile epilogue's global drain)."""

    def _split_multi_waits(self):
        nc = self.nc
        eng_map = {
            mybir.EngineType.PE: nc.tensor,
            mybir.EngineType.DVE: nc.vector,
            mybir.EngineType.Activation: nc.scalar,
            mybir.EngineType.Pool: nc.gpsimd,
            mybir.EngineType.SP: nc.sync,
        }
        tail = nc.cur_bb.bb
        for f in nc.m.functions:
            for bb in f.blocks:
                insts = bb.instructions
                idx = 0
                while idx < len(insts):
                    inst = insts[idx]
                    si = inst.sync_info
                    if si is not None and si.on_wait and len(si.on_wait) > 1:
                        waits = list(si.on_wait)
                        inst.sync_info = mybir.SyncInfo(
                            on_wait=[waits[-1]], on_update=list(si.on_update or [])
                        )
                        for w in waits[:-1]:
                            ni = eng_map[inst.engine].nop(nofuse=True).ins
                            assert tail.instructions[-1] is ni
                            tail.instructions.pop()
                            ni.sync_info = mybir.SyncInfo(on_wait=[w], on_update=[])
                            insts.insert(idx, ni)
                            idx += 1
                    idx += 1

    def _drain_and_barrier(self, tick_clock, wait_clock):
        self._split_multi_waits()
        drain_inst = self.nc.sync.drain()
        wait_clock.add_sem_waits(
            drain_inst.ins, ScopedClock({None: tick_clock.global_clock})
        )
        si = drain_inst.ins.sync_info
        if si is not None and si.on_wait and len(si.on_wait) > 1:
            extra = list(si.on_wait[1:])
            drain_inst.ins.sync_info = mybir.SyncInfo(
                on_wait=[si.on_wait[0]], on_update=list(si.on_update or [])
            )
            for w in extra:
                d2 = self.nc.sync.drain()
                prev = d2.ins.sync_info
                d2.ins.sync_info = mybir.SyncInfo(
                    on_wait=[w],
                    on_update=list(prev.on_update or []) if prev else [],
                )
        self.nc.all_engine_barrier()
        assert self.sems is not None
        popped = self.nc._tile_sem_poison_stack.pop()
        assert popped is self._sem_poison
        self.nc.clear_and_free_semaphores(list(self.sems.allocated().values()))
        self.nc.all_engine_barrier()


def _f32r(ap):
    return ap.bitcast(mybir.dt.float32r)


# ----------------------------------------------------------------------
# L1: QKV projection. Per core: 1024 tokens.
#   qkt [2048, 1024] bf16: rows 0:1024 q^T, 1024:2048 k^T ([feature, token])
#   v   [1024, 1024] bf16: natural [token, feature]
# ----------------------------------------------------------------------
def build_l1():
    nc = bass.Bass()
    f32 = mybir.dt.float32
    bf16 = mybir.dt.bfloat16

    xt = nc.dram_tensor("xt", [8, P, 1024], bf16, kind="ExternalInput")
    wqk = nc.dram_tensor("wqk", [16, P, 8, P], bf16, kind="ExternalInput")
    wv = nc.dram_tensor("wv", [8, P, 1024], bf16, kind="ExternalInput")
    bqk = nc.dram_tensor("bqk", [P, 16], f32, kind="ExternalInput")
    bv = nc.dram_tensor("bv", [1024], f32, kind="ExternalInput")
    qkt = nc.dram_tensor("qkt", [2048, 1024], bf16, kind="ExternalOutput")
    v_nat = nc.dram_tensor("v", [1024, 1024], bf16, kind="ExternalOutput")

    with _TileContext(nc) as tc:
        with (
            tc.tile_pool(name="xres", bufs=1) as xpool,
            tc.tile_pool(name="singles", bufs=1) as singles,
            tc.tile_pool(name="w", bufs=3) as wpool,
            tc.tile_pool(name="out", bufs=3) as opool,
            tc.tile_pool(name="ps", bufs=4, space="PSUM") as pspool,
        ):
            xt_sb = xpool.tile([P, 8, 1024], bf16)
            for et in range(8):
                nc.sync.dma_start(xt_sb[:, et, :], xt[et])

            bqk_sb = singles.tile([P, 16], f32)
            nc.sync.dma_start(bqk_sb, bqk[:, :])
            bv_sb = singles.tile([P, 1024], f32)
            bv_ap = bv[:]
            nc.sync.dma_start(
                bv_sb,
                bass.AP(tensor=bv_ap.tensor, offset=bv_ap.offset, ap=[[0, P], bv_ap.ap[0]]),
            )

            # q^T / k^T: [feature, token]
            for ft in range(16):
                w_sb = wpool.tile([P, 8, P], bf16, tag="wqk")
                nc.sync.dma_start(w_sb, wqk[ft])
                pss = [pspool.tile([P, 512], f32, tag=f"ps{i}", name=f"ps{i}") for i in range(2)]
                for et in range(8):
                    for tcn in range(2):
                        nc.tensor.matmul(
                            pss[tcn],
                            lhsT=w_sb[:, et, :],
                            rhs=xt_sb[:, et, bass.ts(tcn, 512)],
                            start=(et == 0),
                            stop=(et == 7),
                        )
                for tcn in range(2):
                    ot = opool.tile([P, 512], bf16, tag="oqk")
                    nc.vector.tensor_scalar_add(ot, pss[tcn], bqk_sb[:, ft : ft + 1])
                    nc.sync.dma_start(qkt[bass.ts(ft, P), bass.ts(tcn, 512)], ot)

            # v: natural [token, feature]; wv stays resident (2 MiB)
            wv_sb = xpool.tile([P, 8, 1024], bf16)
            for et in range(8):
                nc.sync.dma_start(wv_sb[:, et, :], wv[et])
            for tt in range(8):
                pss = [pspool.tile([P, 512], f32, tag=f"ps{i}", name=f"ps{i}") for i in range(2)]
                for et in range(8):
                    for fc in range(2):
                        nc.tensor.matmul(
                            pss[fc],
                            lhsT=xt_sb[:, et, bass.ts(tt, P)],
                            rhs=wv_sb[:, et, bass.ts(fc, 512)],
                            start=(et == 0),
                            stop=(et == 7),
                        )
                for fc in range(2):
                    ot = opool.tile([P, 512], bf16, tag="ov")
                    nc.vector.tensor_add(ot, pss[fc], bv_sb[:, bass.ts(fc, 512)])
                    nc.sync.dma_start(v_nat[bass.ts(tt, P), bass.ts(fc, 512)], ot)
    return nc


# ----------------------------------------------------------------------
# L2: attention + out-proj + residual + LayerNorm for one variant.
# extents: causal key extents (E1, E2) of the core's two blocks.
# ----------------------------------------------------------------------
def build_l2(extents):
    nc = bass.Bass()
    f32 = mybir.dt.float32
    bf16 = mybir.dt.bfloat16
    KTOT = sum(extents)  # 2560

    qt2 = nc.dram_tensor("qt2", [2, 1024, 1024], bf16, kind="ExternalInput")
    kt2 = nc.dram_tensor("kt2", [1024, KTOT], bf16, kind="ExternalInput")
    vaug = nc.dram_tensor("vaug", [8, KTOT, 144], bf16, kind="ExternalInput")
    xres = nc.dram_tensor("xres", [1024, 1024], f32, kind="ExternalInput")
    wo = nc.dram_tensor("wo", [8, P, 1024], bf16, kind="ExternalInput")
    maskw = nc.dram_tensor("maskw", [P, 896], bf16, kind="ExternalInput")
    onehot = nc.dram_tensor("onehot", [P, 512], bf16, kind="ExternalInput")
    ob = nc.dram_tensor("ob", [1024], f32, kind="ExternalInput")
    gamma = nc.dram_tensor("gamma", [1024], f32, kind="ExternalInput")
    beta = nc.dram_tensor("beta", [1024], f32, kind="ExternalInput")
    out = nc.dram_tensor("out", [1024, 1024], f32, kind="ExternalOutput")

    def bcast(vec):
        ap = vec[:]
        return bass.AP(tensor=ap.tensor, offset=ap.offset, ap=[[0, P], ap.ap[0]])

    with _TileContext(nc) as tc:
        with (
            tc.tile_pool(name="singles", bufs=1) as singles,
            tc.tile_pool(name="kt", bufs=2) as ktp,
            tc.tile_pool(name="qt", bufs=2) as qtp,
            tc.tile_pool(name="va", bufs=2) as vap,
            tc.tile_pool(name="pt", bufs=8) as ptp,
            tc.tile_pool(name="craw", bufs=2) as crawp,
            tc.tile_pool(name="den", bufs=1) as denp,
            tc.tile_pool(name="rec", bufs=1) as recp,
            tc.tile_pool(name="acc", bufs=2) as accp,
            tc.tile_pool(name="xr", bufs=2) as xrp,
            tc.tile_pool(name="st", bufs=4) as stp,
            tc.tile_pool(name="ps_s", bufs=2, space="PSUM") as ps_s,
            tc.tile_pool(name="ps_ctx", bufs=2, space="PSUM") as ps_ctx,
            tc.tile_pool(name="ps_bco", bufs=2, space="PSUM") as ps_bco,
        ):
            mask_sb = singles.tile([P, 896], bf16)
            nc.sync.dma_start(mask_sb, maskw[:, :])
            oh_sb = singles.tile([P, 512], bf16)
            nc.sync.dma_start(oh_sb, onehot[:, :])
            eps_sb = singles.tile([P, 1], f32)
            nc.vector.memset(eps_sb, LN_EPS)
            ob_sb = singles.tile([P, 1024], f32)
            nc.sync.dma_start(ob_sb, bcast(ob))
            gam_sb = singles.tile([P, 1024], f32)
            nc.sync.dma_start(gam_sb, bcast(gamma))
            bet_sb = singles.tile([P, 1024], f32)
            nc.sync.dma_start(bet_sb, bcast(beta))
            wo_sb = []
            for p in range(8):
                wt = singles.tile([P, 1024], bf16, tag=f"wo{p}")
                nc.sync.dma_start(wt, wo[p])
                wo_sb.append(wt)
            # pair-packed normalized context: head 2p at partitions 0:64,
            # head 2p+1 at 64:128 (odd heads arrive via SBUF->SBUF DMA)
            ctxT_all = []
            for p in range(8):
                ct = singles.tile([P, 512], bf16, tag=f"ctxT{p}")
                ctxT_all.append(ct)

            coff = 0
            for blk, EXT in enumerate(extents):
                NT = EXT // P
                ctxT = {}
                craws = {}
                dens = {}
                for pair in range(8):
                    half = pair // 4
                    kt_sb = ktp.tile([P, 2048], bf16, tag="kt")
                    for cc in range(EXT // 512):
                        nc.sync.dma_start(
                            kt_sb[:, bass.ts(cc, 512)],
                            kt2[
                                bass.ts(pair, P),
                                coff + cc * 512 : coff + (cc + 1) * 512,
                            ],
                        )
                    qtA_sb = qtp.tile([P, 512], bf16, tag="qtA")
                    nc.sync.dma_start(
                        qtA_sb, qt2[0, bass.ts(pair, P), bass.ts(blk, 512)]
                    )
                    qtB_sb = qtp.tile([P, 512], bf16, tag="qtB")
                    nc.sync.dma_start(
                        qtB_sb, qt2[1, bass.ts(pair, P), bass.ts(blk, 512)]
                    )
                    va_sb = vap.tile([P, 16, 144], bf16, tag="va")
                    for cc in range(EXT // 512):
                        nc.sync.dma_start(
                            va_sb[:, 4 * cc : 4 * cc + 4, :],
                            vaug[
                                pair,
                                coff + cc * 512 : coff + (cc + 1) * 512,
                                :,
                            ].rearrange("(o p) c -> p o c", p=P),
                        )
                    if pair % 4 == 0:
                        den_sb = denp.tile([72, 512], f32, tag=f"den{half}")
                        nc.vector.memset(den_sb[64:72, :], 0.0)
                        dens[half] = den_sb
                    den_sb = dens[half]
                    for h2 in range(2):
                        h = 2 * pair + h2
                        k8 = h % 8
                        base = 64 * h2
                        ctx_ps = ps_ctx.tile([P, 512], f32)
                        # phase A: scores + exp for the whole extent
                        pts = []
                        for jj in range(NT // 2):
                            s_ps = ps_s.tile([P, 1024], f32)
                            for j2 in range(2):
                                j = 2 * jj + j2
                                nc.tensor.matmul(
                                    s_ps[:, bass.ts(j2, 512)],
                                    lhsT=kt_sb[:, bass.ts(j, P)],
                                    rhs=(qtA_sb if h2 == 0 else qtB_sb),
                                    start=True,
                                    stop=True,
                                )
                            pt = ptp.tile([P, 1024], bf16, tag="pt")
                            nc.scalar.activation(
                                pt, s_ps, mybir.ActivationFunctionType.Exp, scale=0.125
                            )
                            pts.append(pt)
                        # phase B: mask diagonal tiles, then accumulate P.V_aug
                        for jj in range(NT // 2):
                            pt = pts[jj]
                            for j2 in range(2):
                                j = 2 * jj + j2
                                if j >= NT - 4:
                                    off = 384 - P * (j - (NT - 4))
                                    nc.vector.tensor_mul(
                                        pt[:, bass.ts(j2, 512)],
                                        pt[:, bass.ts(j2, 512)],
                                        mask_sb[:, off : off + 512],
                                    )
                        for jj in range(NT // 2):
                            pt = pts[jj]
                            for j2 in range(2):
                                j = 2 * jj + j2
                                nc.tensor.matmul(
                                    ctx_ps[0:72, :],
                                    lhsT=va_sb[:, j, 72 * h2 : 72 * h2 + 72],
                                    rhs=pt[:, bass.ts(j2, 512)],
                                    start=(j == 0),
                                    stop=(j == NT - 1),
                                    skip_group_check=True,
                                )
                        # stash unnormalized ctx + denominator row, free psum
                        craw = crawp.tile([64, 512], f32, tag=f"craw{h % 8}")
                        nc.vector.tensor_copy(craw, ctx_ps[0:64, :])
                        craws[h] = craw
                        # rows 64:72 of ctx_ps are zero except row 64+k8
                        # (one-hot aug), so an aligned 8-row add accumulates
                        # exactly this head's denominator into its slot.
                        nc.vector.tensor_add(
                            den_sb[64:72, :], den_sb[64:72, :], ctx_ps[64:72, :]
                        )
                    def _normalize_batch(half, craw_snapshot):
                        # batched reciprocal for 8 heads, then bcast+scale
                        rec = recp.tile([P, 512], bf16, tag=f"rec{half}")
                        nc.vector.memset(rec, 0.0)
                        with nc.allow_low_precision(reason="softmax denom in bf16"):
                            nc.vector.reciprocal(
                                rec[64:72, :], dens[half][64:72, :]
                            )
                        for hh in range(8):
                            h = 8 * half + hh
                            bc_ps = ps_bco.tile([P, 512], f32, tag="bco")
                            nc.tensor.matmul(
                                bc_ps[0:64, :],
                                lhsT=oh_sb[:, bass.ts(hh, 64)],
                                rhs=rec,
                                start=True,
                                stop=True,
                            )
                            ct = ctxT_all[h // 2]
                            if h % 2 == 0:
                                nc.vector.tensor_mul(
                                    ct[0:64, :], craw_snapshot[h], bc_ps[0:64, :]
                                )
                            else:
                                otmp = recp.tile([64, 512], bf16, tag="otmp")
                                nc.vector.tensor_mul(
                                    otmp, craw_snapshot[h], bc_ps[0:64, :]
                                )
                                nc.gpsimd.dma_start(ct[64:128, :], otmp)
                            ctxT[h] = ct

                    if pair == 4:
                        # deferred: emit batch-0 normalize after pair 4's
                        # attention so its bcasts don't stall the PE stream
                        _normalize_batch(0, dict(craws))
                    elif pair == 7:
                        _normalize_batch(1, dict(craws))

                # out-proj + residual + LN for this block
                for tt in range(4):
                    acc = accp.tile([P, 1024], f32)
                    o_pss = [
                        ps_bco.tile([P, 512], f32, tag="bco", name=f"ops{i}")
                        for i in range(2)
                    ]
                    for p in range(8):
                        for ec in range(2):
                            nc.tensor.matmul(
                                o_pss[ec],
                                lhsT=ctxT_all[p][:, bass.ts(tt, P)],
                                rhs=wo_sb[p][:, bass.ts(ec, 512)],
                                start=(p == 0),
                                stop=(p == 7),
                            )
                    for ec in range(2):
                        nc.vector.tensor_add(
                            acc[:, bass.ts(ec, 512)],
                            o_pss[ec],
                            ob_sb[:, bass.ts(ec, 512)],
                        )
                    xr = xrp.tile([P, 1024], f32)
                    nc.gpsimd.dma_start(
                        xr, xres[blk * 512 + tt * P : blk * 512 + (tt + 1) * P, :]
                    )
                    nc.vector.tensor_add(acc, acc, xr)
                    stats = stp.tile([P, 2, 6], f32, tag="stats")
                    nc.vector.bn_stats(stats[:, 0, :], acc[:, 0:512])
                    nc.vector.bn_stats(stats[:, 1, :], acc[:, 512:1024])
                    mv = stp.tile([P, 2], f32, tag="mv")
                    nc.vector.bn_aggr(mv, stats)
                    std = stp.tile([P, 1], f32, tag="std")
                    nc.scalar.activation(
                        std,
                        mv[:, 1:2],
                        mybir.ActivationFunctionType.Sqrt,
                        bias=eps_sb,
                        scale=1.0,
                    )
                    nc.vector.reciprocal(std, std)
                    nc.vector.tensor_scalar(
                        acc,
                        acc,
                        scalar1=mv[:, 0:1],
                        scalar2=std,
                        op0=mybir.AluOpType.subtract,
                        op1=mybir.AluOpType.mult,
                    )
                    nc.vector.tensor_mul(acc, acc, gam_sb)
                    nc.vector.tensor_add(acc, acc, bet_sb)
                    nc.sync.dma_start(
                        out[blk * 512 + tt * P : blk * 512 + (tt + 1) * P, :], acc
                    )
                coff += EXT
    return nc


# ----------------------------------------------------------------------
# Dispatch: L1 via run_bass_kernel_spmd (uniform); L2 via two concurrent
# shard_map programs on disjoint 4-device submeshes.
# ----------------------------------------------------------------------
_CACHE = {}
PROFILE = False
LAST_TIMING = {}


def _ensure_hook_shim():
    """Register antenv.axon_hooks (missing in this image) so that
    bass_utils' trace path and our own L2 profiling can drive NTFF
    capture through libaxon_pjrt.so."""
    import types
    import ctypes
    import contextlib

    try:
        from antenv.axon_hooks import get_axon_ntff_profile_hook  # noqa: F401
        return True
    except ImportError:
        pass
    so_path = "/opt/axon/libaxon_pjrt.so"
    if not os.path.exists(so_path):
        return False
    lib = ctypes.CDLL(so_path)
    if not hasattr(lib, "axon_start_nrt_profile"):
        return False
    lib.axon_start_nrt_profile.argtypes = [
        ctypes.POINTER(ctypes.c_int64),
        ctypes.c_size_t,
    ]
    lib.axon_start_nrt_profile.restype = ctypes.c_int64
    lib.axon_stop_nrt_profile.argtypes = [ctypes.c_char_p]
    lib.axon_stop_nrt_profile.restype = ctypes.c_int64

    @contextlib.contextmanager
    def _hook(output_dir, device_ids):
        import jax

        jax.devices()
        if device_ids:
            ids = (ctypes.c_int64 * len(device_ids))(*device_ids)
            rc = lib.axon_start_nrt_profile(ids, len(device_ids))
        else:
            rc = lib.axon_start_nrt_profile(None, 0)
        if rc != 0:
            raise RuntimeError(f"axon_start_nrt_profile rc={rc}")
        try:
            yield
        finally:
            n = lib.axon_stop_nrt_profile(str(output_dir).encode())
            if n < 0:
                raise RuntimeError(f"axon_stop_nrt_profile rc={n}")

    import antenv

    mod = types.ModuleType("antenv.axon_hooks")
    _state = {"hook": _hook}
    mod.get_axon_ntff_profile_hook = lambda: _state["hook"]
    mod.set_axon_ntff_profile_hook = lambda h: _state.__setitem__("hook", h)
    sys.modules["antenv.axon_hooks"] = mod
    antenv.axon_hooks = mod
    return True


def _process_l2_profile(nc, neff_dir, label, model_idx=0):
    import glob as _glob
    import gauge.profiler
    from concourse.bass_utils import _process_ntff_profile
    from concourse._compat import FishPath

    ntffs = _glob.glob(os.path.join(neff_dir, "*_body*.ntff"))
    if not ntffs:
        LAST_TIMING[label] = None
        return
    profile = gauge.profiler.Profile(
        profile_path=FishPath(neff_dir),
        kernel_dev_mode=True,
        profile_on_exit=False,
        bass_kernel=nc.m,
        offline_processing=True,
        fname="*_body*",
        metadata={},
    )
    perf = _process_ntff_profile(
        profile, neff_dir, nc, [model_idx], [model_idx], False, {"title": label}, False
    )
    LAST_TIMING[label] = perf.exec_time_ns
    LAST_TIMING[label + "_trace"] = perf.insts_and_trace_path


def _l2_exec(nc, n_cores):
    """Build a jitted shard_map executor for one L2 variant (modeled on
    bass2jax.run_bass_via_pjrt's multi-core branch, but device-list aware)."""
    import jax
    from jax.experimental.shard_map import shard_map
    from jax.sharding import Mesh, PartitionSpec
    from concourse import bass2jax

    bass2jax.install_neuronx_cc_hook()

    partition_name = nc.partition_id_tensor.name if nc.partition_id_tensor else None
    in_names, out_names, out_avals, zero_shapes = [], [], [], []
    for alloc in nc.m.functions[0].allocations:
        if not isinstance(alloc, mybir.MemoryLocationSet):
            continue
        name = alloc.memorylocations[0].name
        if alloc.kind == "ExternalInput":
            if name != partition_name:
                in_names.append(name)
        elif alloc.kind == "ExternalOutput":
            out_names.append(name)
            shape = tuple(alloc.tensor_shape)
            dtype = mybir.dt.np(alloc.dtype)
            out_avals.append(jax.core.ShapedArray(shape, dtype))
            zero_shapes.append((shape, dtype))
    n_params = len(in_names)
    all_names = in_names + out_names
    if partition_name is not None:
        all_names = all_names + [partition_name]
    donate = tuple(range(n_params, n_params + len(out_names)))

    def _body(*args):
        operands = list(args)
        if partition_name is not None:
            operands.append(bass2jax.partition_id_tensor())
        outs = bass2jax._bass_exec_p.bind(
            *operands,
            out_avals=tuple(out_avals),
            in_names=tuple(all_names),
            out_names=tuple(out_names),
            lowering_input_output_aliases=(),
            sim_require_finite=True,
            sim_require_nnan=True,
            nc=nc,
        )
        return tuple(outs)

    def run(in_maps, devices):
        assert len(in_maps) == n_cores
        mesh = Mesh(np.asarray(devices), ("core",))
        specs = (PartitionSpec("core"),) * (n_params + len(out_names))
        sharded = jax.jit(
            shard_map(
                _body,
                mesh=mesh,
                in_specs=specs,
                out_specs=(PartitionSpec("core"),) * len(out_names),
                check_rep=False,
            ),
            donate_argnums=donate,
            keep_unused=True,
        )
        concat_in = [
            np.concatenate([np.asarray(m[nm]) for m in in_maps], axis=0)
            for nm in in_names
        ]
        concat_zero = [
            np.zeros((n_cores * sh[0], *sh[1:]), dt) for sh, dt in zero_shapes
        ]
        return sharded(*concat_in, *concat_zero), out_names, out_avals

    return run


def _get_programs():
    if "l1" not in _CACHE:
        _CACHE["l1"] = build_l1()
        _CACHE["l2A"] = build_l2(VARIANT_A)
        _CACHE["l2B"] = build_l2(VARIANT_B)
        _CACHE["runA"] = _l2_exec(_CACHE["l2A"], 4)
        _CACHE["runB"] = _l2_exec(_CACHE["l2B"], 4)
    return _CACHE


def _block_rows(g):
    b, k = g // 4, g % 4
    return b, 512 * k, 512 * (k + 1)


def kernel(txt_emb, in_proj_w, in_proj_b, out_proj_w, out_proj_b, ln_gamma, ln_beta):
    import jax
    from concourse.bass_utils import run_bass_kernel_spmd

    progs = _get_programs()

    x = np.asarray(txt_emb, dtype=np.float32)
    wi = np.asarray(in_proj_w, dtype=np.float32)
    bi = np.asarray(in_proj_b, dtype=np.float32)
    wob = np.asarray(out_proj_w, dtype=np.float32)
    obb = np.asarray(out_proj_b, dtype=np.float32)
    gam = np.asarray(ln_gamma, dtype=np.float32)
    bet = np.asarray(ln_beta, dtype=np.float32)

    # ---- host prep (layouts) ----
    wT = np.ascontiguousarray(wi.T)  # [E, 3E]
    wqk_t = np.ascontiguousarray(
        wT[:, :2048].reshape(8, P, 16, P).transpose(2, 1, 0, 3)
    ).astype(BF16)  # [16, 128, 8, 128] = [ft, e_in, e_outer, f_in]
    wv_t = np.ascontiguousarray(wT[:, 2048:].reshape(8, P, 1024)).astype(BF16)
    bqk = np.ascontiguousarray(bi[:2048].reshape(16, P).T)  # [128, 16]
    bv = np.ascontiguousarray(bi[2048:])

    core_tokens = []  # per core [1024, 1024] natural x
    for c in range(8):
        rows = []
        for g in CORE_BLOCKS[c]:
            b, r0, r1 = _block_rows(g)
            rows.append(x[b, r0:r1, :])
        core_tokens.append(np.concatenate(rows, axis=0))

    l1_maps = []
    for c in range(8):
        xt = np.ascontiguousarray(core_tokens[c].T.reshape(8, P, 1024)).astype(BF16)
        l1_maps.append(
            {"xt": xt, "wqk": wqk_t, "wv": wv_t, "bqk": bqk, "bv": bv}
        )

    if PROFILE:
        _ensure_hook_shim()
        res1 = run_bass_kernel_spmd(
            progs["l1"], l1_maps, list(range(8)), trace=True
        )
        LAST_TIMING["l1"] = res1.exec_time_ns
        LAST_TIMING["l1_trace"] = res1.instructions_and_trace
    else:
        res1 = run_bass_kernel_spmd(progs["l1"], l1_maps, list(range(8)))

    # ---- assemble global K^T and V-aug ----
    KT = np.empty((B, 1024, 2048), dtype=BF16)
    VA = np.zeros((B, 2048, H, 72), dtype=BF16)
    for h in range(H):
        VA[:, :, h, 64 + h % 8] = np.asarray(1.0, dtype=BF16)
    QT = []  # per-core q^T [1024, 1024]
    for c in range(8):
        qkt = res1.results[c]["qkt"]
        vna = res1.results[c]["v"]
        QT.append(qkt[:1024])
        for slot, g in enumerate(CORE_BLOCKS[c]):
            b, r0, r1 = _block_rows(g)
            KT[b][:, r0:r1] = qkt[1024:, 512 * slot : 512 * (slot + 1)]
            VA[b, r0:r1, :, :64] = (
                vna[512 * slot : 512 * (slot + 1)].reshape(512, H, 64)
            )

    # ---- per-core L2 inputs ----
    mask = np.zeros((P, 896), dtype=BF16)
    yy = np.arange(896)[None, :]
    pp = np.arange(P)[:, None]
    mask[yy >= pp + 384] = np.asarray(1.0, dtype=BF16)
    onehot = np.zeros((P, 512), dtype=BF16)
    for hh in range(8):
        onehot[64 + hh, 64 * hh : 64 * hh + 64] = np.asarray(1.0, dtype=BF16)
    wo_t = np.ascontiguousarray(
        wob.T.reshape(8, P, 1024)
    ).astype(BF16)  # pair p rows: head 2p then head 2p+1

    l2_maps = {}
    for c in range(8):
        exts = _extents(c)
        kt_parts, va_parts = [], []
        for slot, g in enumerate(CORE_BLOCKS[c]):
            b, _, _ = _block_rows(g)
            kt_parts.append(KT[b][:, : exts[slot]])
            va_parts.append(VA[b][: exts[slot]])
        kt2 = np.ascontiguousarray(np.concatenate(kt_parts, axis=1))
        va = np.concatenate(va_parts, axis=0)  # [2560, 16, 72]
        va = np.ascontiguousarray(
            va.reshape(sum(exts), 8, 144).transpose(1, 0, 2)
        )
        qtz = np.zeros((2, 1024, 1024), dtype=BF16)
        qf = np.arange(1024)
        even = (qf // 64) % 2 == 0
        qtz[0, even] = QT[c][even]
        qtz[1, ~even] = QT[c][~even]
        l2_maps[c] = {
            "qt2": qtz,
            "kt2": kt2,
            "vaug": va,
            "xres": np.ascontiguousarray(core_tokens[c]),
            "wo": wo_t,
            "maskw": mask,
            "onehot": onehot,
            "ob": obb,
            "gamma": gam,
            "beta": bet,
        }

    devices = jax.devices()
    coresA = CORES_OF_VARIANT[VARIANT_A]
    coresB = CORES_OF_VARIANT[VARIANT_B]
    if PROFILE:
        import tempfile
        from antenv.axon_hooks import get_axon_ntff_profile_hook

        _ensure_hook_shim()
        from antenv.axon_hooks import get_axon_ntff_profile_hook

        hook = get_axon_ntff_profile_hook()
        dirA = tempfile.mkdtemp(prefix="l2A_")
        with hook(dirA, [devices[coresA[0]].id]):
            outA, namesA, avalsA = progs["runA"](
                [l2_maps[c] for c in coresA], [devices[c] for c in coresA]
            )
            jax.block_until_ready(outA)
        _process_l2_profile(progs["l2A"], dirA, "l2A", devices[coresA[0]].id)
        dirB = tempfile.mkdtemp(prefix="l2B_")
        with hook(dirB, [devices[coresB[0]].id]):
            outB, namesB, avalsB = progs["runB"](
                [l2_maps[c] for c in coresB], [devices[c] for c in coresB]
            )
            jax.block_until_ready(outB)
        _process_l2_profile(progs["l2B"], dirB, "l2B", devices[coresB[0]].id)
    else:
        outA, namesA, avalsA = progs["runA"](
            [l2_maps[c] for c in coresA], [devices[c] for c in coresA]
        )
        outB, namesB, avalsB = progs["runB"](
            [l2_maps[c] for c in coresB], [devices[c] for c in coresB]
        )

    result = np.empty((B, S, E), dtype=np.float32)

    def scatter(out_arrs, names, avals, cores):
        arr = np.asarray(out_arrs[names.index("out")]).reshape(
            len(cores), *avals[names.index("out")].shape
        )
        for ci, c in enumerate(cores):
            for slot, g in enumerate(CORE_BLOCKS[c]):
                b, r0, r1 = _block_rows(g)
                result[b, r0:r1, :] = arr[ci, 512 * slot : 512 * (slot + 1), :]

    scatter(outA, namesA, avalsA, coresA)
    scatter(outB, namesB, avalsB, coresB)
    return result


# revision 30
# speedup vs baseline: 1.0156x; 1.0156x over previous
"""Causal self-attention block (QKV proj -> 16-head causal attention ->
out proj -> residual + LayerNorm) on 8 Trainium2 NeuronCores.

Sharding: the 4x2048 tokens are split into 16 blocks of 512; core i owns
blocks {i, 15-i}, which balances causal-attention work exactly (every
core attends to 2560 key-tokens total). Two SPMD launches:
  L1: QKV projection (uniform program, 8 cores). Emits q^T/k^T in
      [feature, token] layout and V in natural [token, feature] layout.
  L2: attention + out-proj + LayerNorm. Scores are computed transposed
      (S^T[tk, tq]) so softmax needs no partition reduction: exp runs on
      ScalarE (no max-subtraction needed -- scores are O(1) here), and
      the denominator falls out of a ones-column appended to V. Two
      program variants (block extents (512, 2048) and (1024, 1536))
      dispatched concurrently on disjoint 4-device submeshes.
Matmuls run in bf16 (1 cycle/row) except the QKV projection which uses
float32r (full fp32 at 1 cycle/row for N>=512).
"""

import os
import sys

for _p in ("/opt/trn_rl_repo",):
    if os.path.isdir(_p) and _p not in sys.path:
        sys.path.insert(0, _p)

import numpy as np
import ml_dtypes

import concourse.bass as bass
import concourse.mybir as mybir
import concourse.tile as tile
from concourse.vector_clock import ScopedClock

BF16 = ml_dtypes.bfloat16

B, S, E = 4, 2048, 1024
H, D = 16, 64
P = 128
BLK = 512  # token block
NBLK = 16  # global 512-token blocks
LN_EPS = 1e-5

# core i owns global blocks {i, 15-i}; process smaller causal extent first.
CORE_BLOCKS = []
for i in range(8):
    g1, g2 = i, 15 - i
    bl = sorted((g1, g2), key=lambda g: g % 4)
    CORE_BLOCKS.append(tuple(bl))


def _extents(core):
    return tuple((g % 4 + 1) * BLK for g in CORE_BLOCKS[core])


VARIANT_A = (512, 2048)  # cores 0,3,4,7
VARIANT_B = (1024, 1536)  # cores 1,2,5,6
CORES_OF_VARIANT = {
    VARIANT_A: [c for c in range(8) if _extents(c) == VARIANT_A],
    VARIANT_B: [c for c in range(8) if _extents(c) == VARIANT_B],
}


class _TileContext(tile.TileContext):
    """This walrus build rejects >1 sync-wait per instruction; move extra
    waits onto preceding same-engine NOPs (and extra chained drains for
    the Tile epilogue's global drain)."""

    def _split_multi_waits(self):
        nc = self.nc
        eng_map = {
            mybir.EngineType.PE: nc.tensor,
            mybir.EngineType.DVE: nc.vector,
            mybir.EngineType.Activation: nc.scalar,
            mybir.EngineType.Pool: nc.gpsimd,
            mybir.EngineType.SP: nc.sync,
        }
        tail = nc.cur_bb.bb
        for f in nc.m.functions:
            for bb in f.blocks:
                insts = bb.instructions
                idx = 0
                while idx < len(insts):
                    inst = insts[idx]
                    si = inst.sync_info
                    if si is not None and si.on_wait and len(si.on_wait) > 1:
                        waits = list(si.on_wait)
                        inst.sync_info = mybir.SyncInfo(
                            on_wait=[waits[-1]], on_update=list(si.on_update or [])
                        )
                        for w in waits[:-1]:
                            ni = eng_map[inst.engine].nop(nofuse=True).ins
                            assert tail.instructions[-1] is ni
                            tail.instructions.pop()
                            ni.sync_info = mybir.SyncInfo(on_wait=[w], on_update=[])
                            insts.insert(idx, ni)
                            idx += 1
                    idx += 1

    def _drain_and_barrier(self, tick_clock, wait_clock):
        self._split_multi_waits()
        drain_inst = self.nc.sync.drain()
        wait_clock.add_sem_waits(
            drain_inst.ins, ScopedClock({None: tick_clock.global_clock})
        )
        si = drain_inst.ins.sync_info
        if si is not None and si.on_wait and len(si.on_wait) > 1:
            extra = list(si.on_wait[1:])
            drain_inst.ins.sync_info = mybir.SyncInfo(
                on_wait=[si.on_wait[0]], on_update=list(si.on_update or [])
            )
            for w in extra:
                d2 = self.nc.sync.drain()
                prev = d2.ins.sync_info
                d2.ins.sync_info = mybir.SyncInfo(
                    on_wait=[w],
                    on_update=list(prev.on_update or []) if prev else [],
                )
        self.nc.all_engine_barrier()
        assert self.sems is not None
        popped = self.nc._tile_sem_poison_stack.pop()
        assert popped is self._sem_poison
        self.nc.clear_and_free_semaphores(list(self.sems.allocated().values()))
        self.nc.all_engine_barrier()


def _f32r(ap):
    return ap.bitcast(mybir.dt.float32r)


# ----------------------------------------------------------------------
# L1: QKV projection. Per core: 1024 tokens.
#   qkt [2048, 1024] bf16: rows 0:1024 q^T, 1024:2048 k^T ([feature, token])
#   v   [1024, 1024] bf16: natural [token, feature]
# ----------------------------------------------------------------------
def build_l1():
    nc = bass.Bass()
    f32 = mybir.dt.float32
    bf16 = mybir.dt.bfloat16

    xt = nc.dram_tensor("xt", [8, P, 1024], bf16, kind="ExternalInput")
    wqk = nc.dram_tensor("wqk", [16, P, 8, P], bf16, kind="ExternalInput")
    wv = nc.dram_tensor("wv", [8, P, 1024], bf16, kind="ExternalInput")
    bqk = nc.dram_tensor("bqk", [P, 16], f32, kind="ExternalInput")
    bv = nc.dram_tensor("bv", [1024], f32, kind="ExternalInput")
    qkt = nc.dram_tensor("qkt", [2048, 1024], bf16, kind="ExternalOutput")
    v_nat = nc.dram_tensor("v", [1024, 1024], bf16, kind="ExternalOutput")

    with _TileContext(nc) as tc:
        with (
            tc.tile_pool(name="xres", bufs=1) as xpool,
            tc.tile_pool(name="singles", bufs=1) as singles,
            tc.tile_pool(name="w", bufs=3) as wpool,
            tc.tile_pool(name="out", bufs=3) as opool,
            tc.tile_pool(name="ps", bufs=4, space="PSUM") as pspool,
        ):
            xt_sb = xpool.tile([P, 8, 1024], bf16)
            for et in range(8):
                nc.sync.dma_start(xt_sb[:, et, :], xt[et])

            bqk_sb = singles.tile([P, 16], f32)
            nc.sync.dma_start(bqk_sb, bqk[:, :])
            bv_sb = singles.tile([P, 1024], f32)
            bv_ap = bv[:]
            nc.sync.dma_start(
                bv_sb,
                bass.AP(tensor=bv_ap.tensor, offset=bv_ap.offset, ap=[[0, P], bv_ap.ap[0]]),
            )

            # q^T / k^T: [feature, token]
            for ft in range(16):
                w_sb = wpool.tile([P, 8, P], bf16, tag="wqk")
                nc.sync.dma_start(w_sb, wqk[ft])
                pss = [pspool.tile([P, 512], f32, tag=f"ps{i}", name=f"ps{i}") for i in range(2)]
                for et in range(8):
                    for tcn in range(2):
                        nc.tensor.matmul(
                            pss[tcn],
                            lhsT=w_sb[:, et, :],
                            rhs=xt_sb[:, et, bass.ts(tcn, 512)],
                            start=(et == 0),
                            stop=(et == 7),
                        )
                for tcn in range(2):
                    ot = opool.tile([P, 512], bf16, tag="oqk")
                    nc.vector.tensor_scalar_add(ot, pss[tcn], bqk_sb[:, ft : ft + 1])
                    nc.sync.dma_start(qkt[bass.ts(ft, P), bass.ts(tcn, 512)], ot)

            # v: natural [token, feature]; wv stays resident (2 MiB)
            wv_sb = xpool.tile([P, 8, 1024], bf16)
            for et in range(8):
                nc.sync.dma_start(wv_sb[:, et, :], wv[et])
            for tt in range(8):
                pss = [pspool.tile([P, 512], f32, tag=f"ps{i}", name=f"ps{i}") for i in range(2)]
                for et in range(8):
                    for fc in range(2):
                        nc.tensor.matmul(
                            pss[fc],
                            lhsT=xt_sb[:, et, bass.ts(tt, P)],
                            rhs=wv_sb[:, et, bass.ts(fc, 512)],
                            start=(et == 0),
                            stop=(et == 7),
                        )
                for fc in range(2):
                    ot = opool.tile([P, 512], bf16, tag="ov")
                    nc.vector.tensor_add(ot, pss[fc], bv_sb[:, bass.ts(fc, 512)])
                    nc.sync.dma_start(v_nat[bass.ts(tt, P), bass.ts(fc, 512)], ot)
    return nc


# ----------------------------------------------------------------------
# L2: attention + out-proj + residual + LayerNorm for one variant.
# extents: causal key extents (E1, E2) of the core's two blocks.
# ----------------------------------------------------------------------
def build_l2(extents):
    nc = bass.Bass()
    f32 = mybir.dt.float32
    bf16 = mybir.dt.bfloat16
    KTOT = sum(extents)  # 2560

    qt2 = nc.dram_tensor("qt2", [2, 1024, 1024], bf16, kind="ExternalInput")
    kt2 = nc.dram_tensor("kt2", [1024, KTOT], bf16, kind="ExternalInput")
    vaug = nc.dram_tensor("vaug", [8, KTOT, 144], bf16, kind="ExternalInput")
    xres = nc.dram_tensor("xres", [1024, 1024], f32, kind="ExternalInput")
    wo = nc.dram_tensor("wo", [8, P, 1024], bf16, kind="ExternalInput")
    maskw = nc.dram_tensor("maskw", [P, 896], bf16, kind="ExternalInput")
    onehot = nc.dram_tensor("onehot", [P, 512], bf16, kind="ExternalInput")
    ob = nc.dram_tensor("ob", [1024], f32, kind="ExternalInput")
    gamma = nc.dram_tensor("gamma", [1024], f32, kind="ExternalInput")
    beta = nc.dram_tensor("beta", [1024], f32, kind="ExternalInput")
    out = nc.dram_tensor("out", [1024, 1024], f32, kind="ExternalOutput")

    def bcast(vec):
        ap = vec[:]
        return bass.AP(tensor=ap.tensor, offset=ap.offset, ap=[[0, P], ap.ap[0]])

    with _TileContext(nc) as tc:
        with (
            tc.tile_pool(name="singles", bufs=1) as singles,
            tc.tile_pool(name="kt", bufs=2) as ktp,
            tc.tile_pool(name="qt", bufs=2) as qtp,
            tc.tile_pool(name="va", bufs=2) as vap,
            tc.tile_pool(name="pt", bufs=8) as ptp,
            tc.tile_pool(name="craw", bufs=2) as crawp,
            tc.tile_pool(name="den", bufs=1) as denp,
            tc.tile_pool(name="rec", bufs=1) as recp,
            tc.tile_pool(name="acc", bufs=2) as accp,
            tc.tile_pool(name="xr", bufs=2) as xrp,
            tc.tile_pool(name="st", bufs=4) as stp,
            tc.tile_pool(name="ps_s", bufs=2, space="PSUM") as ps_s,
            tc.tile_pool(name="ps_ctx", bufs=2, space="PSUM") as ps_ctx,
            tc.tile_pool(name="ps_bco", bufs=2, space="PSUM") as ps_bco,
        ):
            mask_sb = singles.tile([P, 896], bf16)
            nc.sync.dma_start(mask_sb, maskw[:, :])
            oh_sb = singles.tile([P, 512], bf16)
            nc.gpsimd.dma_start(oh_sb, onehot[:, :])
            eps_sb = singles.tile([P, 1], f32)
            nc.vector.memset(eps_sb, LN_EPS)
            ob_sb = singles.tile([P, 1024], f32)
            nc.gpsimd.dma_start(ob_sb, bcast(ob))
            gam_sb = singles.tile([P, 1024], f32)
            nc.gpsimd.dma_start(gam_sb, bcast(gamma))
            bet_sb = singles.tile([P, 1024], f32)
            nc.gpsimd.dma_start(bet_sb, bcast(beta))
            wo_sb = []
            for p in range(8):
                wt = singles.tile([P, 1024], bf16, tag=f"wo{p}")
                nc.gpsimd.dma_start(wt, wo[p])
                wo_sb.append(wt)
            # pair-packed normalized context: head 2p at partitions 0:64,
            # head 2p+1 at 64:128 (odd heads arrive via SBUF->SBUF DMA)
            ctxT_all = []
            for p in range(8):
                ct = singles.tile([P, 512], bf16, tag=f"ctxT{p}")
                ctxT_all.append(ct)

            coff = 0
            for blk, EXT in enumerate(extents):
                NT = EXT // P
                ctxT = {}
                craws = {}
                dens = {}
                for pair in range(8):
                    half = pair // 4
                    kt_sb = ktp.tile([P, 2048], bf16, tag="kt")
                    for cc in range(EXT // 512):
                        nc.sync.dma_start(
                            kt_sb[:, bass.ts(cc, 512)],
                            kt2[
                                bass.ts(pair, P),
                                coff + cc * 512 : coff + (cc + 1) * 512,
                            ],
                        )
                    qtA_sb = qtp.tile([P, 512], bf16, tag="qtA")
                    nc.sync.dma_start(
                        qtA_sb, qt2[0, bass.ts(pair, P), bass.ts(blk, 512)]
                    )
                    qtB_sb = qtp.tile([P, 512], bf16, tag="qtB")
                    nc.sync.dma_start(
                        qtB_sb, qt2[1, bass.ts(pair, P), bass.ts(blk, 512)]
                    )
                    va_sb = vap.tile([P, 16, 144], bf16, tag="va")
                    for cc in range(EXT // 512):
                        nc.sync.dma_start(
                            va_sb[:, 4 * cc : 4 * cc + 4, :],
                            vaug[
                                pair,
                                coff + cc * 512 : coff + (cc + 1) * 512,
                                :,
                            ].rearrange("(o p) c -> p o c", p=P),
                        )
                    if pair % 4 == 0:
                        den_sb = denp.tile([72, 512], f32, tag=f"den{half}")
                        nc.vector.memset(den_sb[64:72, :], 0.0)
                        dens[half] = den_sb
                    den_sb = dens[half]
                    for h2 in range(2):
                        h = 2 * pair + h2
                        k8 = h % 8
                        base = 64 * h2
                        ctx_ps = ps_ctx.tile([P, 512], f32)
                        # phase A: scores + exp for the whole extent
                        pts = []
                        for jj in range(NT // 2):
                            s_ps = ps_s.tile([P, 1024], f32)
                            for j2 in range(2):
                                j = 2 * jj + j2
                                nc.tensor.matmul(
                                    s_ps[:, bass.ts(j2, 512)],
                                    lhsT=kt_sb[:, bass.ts(j, P)],
                                    rhs=(qtA_sb if h2 == 0 else qtB_sb),
                                    start=True,
                                    stop=True,
                                )
                            pt = ptp.tile([P, 1024], bf16, tag="pt")
                            nc.scalar.activation(
                                pt, s_ps, mybir.ActivationFunctionType.Exp, scale=0.125
                            )
                            pts.append(pt)
                        # phase B: mask diagonal tiles, then accumulate P.V_aug
                        for jj in range(NT // 2):
                            pt = pts[jj]
                            for j2 in range(2):
                                j = 2 * jj + j2
                                if j >= NT - 4:
                                    off = 384 - P * (j - (NT - 4))
                                    nc.vector.tensor_mul(
                                        pt[:, bass.ts(j2, 512)],
                                        pt[:, bass.ts(j2, 512)],
                                        mask_sb[:, off : off + 512],
                                    )
                        for jj in range(NT // 2):
                            pt = pts[jj]
                            for j2 in range(2):
                                j = 2 * jj + j2
                                nc.tensor.matmul(
                                    ctx_ps[0:72, :],
                                    lhsT=va_sb[:, j, 72 * h2 : 72 * h2 + 72],
                                    rhs=pt[:, bass.ts(j2, 512)],
                                    start=(j == 0),
                                    stop=(j == NT - 1),
                                    skip_group_check=True,
                                )
                        # stash unnormalized ctx + denominator row, free psum
                        craw = crawp.tile([64, 512], f32, tag=f"craw{h % 8}")
                        nc.vector.tensor_copy(craw, ctx_ps[0:64, :])
                        craws[h] = craw
                        # rows 64:72 of ctx_ps are zero except row 64+k8
                        # (one-hot aug), so an aligned 8-row add accumulates
                        # exactly this head's denominator into its slot.
                        nc.vector.tensor_add(
                            den_sb[64:72, :], den_sb[64:72, :], ctx_ps[64:72, :]
                        )
                    def _normalize_batch(half, craw_snapshot):
                        # batched reciprocal for 8 heads, then bcast+scale
                        rec = recp.tile([P, 512], bf16, tag=f"rec{half}")
                        nc.vector.memset(rec, 0.0)
                        with nc.allow_low_precision(reason="softmax denom in bf16"):
                            nc.vector.reciprocal(
                                rec[64:72, :], dens[half][64:72, :]
                            )
                        for hh in range(8):
                            h = 8 * half + hh
                            bc_ps = ps_bco.tile([P, 512], f32, tag="bco")
                            nc.tensor.matmul(
                                bc_ps[0:64, :],
                                lhsT=oh_sb[:, bass.ts(hh, 64)],
                                rhs=rec,
                                start=True,
                                stop=True,
                            )
                            ct = ctxT_all[h // 2]
                            if h % 2 == 0:
                                nc.vector.tensor_mul(
                                    ct[0:64, :], craw_snapshot[h], bc_ps[0:64, :]
                                )
                            else:
                                otmp = recp.tile([64, 512], bf16, tag="otmp")
                                nc.vector.tensor_mul(
                                    otmp, craw_snapshot[h], bc_ps[0:64, :]
                                )
                                nc.gpsimd.dma_start(ct[64:128, :], otmp)
                            ctxT[h] = ct

                    if pair == 4:
                        # deferred: emit batch-0 normalize after pair 4's
                        # attention so its bcasts don't stall the PE stream
                        _normalize_batch(0, dict(craws))
                    elif pair == 7:
                        _normalize_batch(1, dict(craws))

                # out-proj + residual + LN for this block
                for tt in range(4):
                    acc = accp.tile([P, 1024], f32)
                    o_pss = [
                        ps_bco.tile([P, 512], f32, tag="bco", name=f"ops{i}")
                        for i in range(2)
                    ]
                    for p in range(8):
                        for ec in range(2):
                            nc.tensor.matmul(
                                o_pss[ec],
                                lhsT=ctxT_all[p][:, bass.ts(tt, P)],
                                rhs=wo_sb[p][:, bass.ts(ec, 512)],
                                start=(p == 0),
                                stop=(p == 7),
                            )
                    for ec in range(2):
                        nc.vector.tensor_add(
                            acc[:, bass.ts(ec, 512)],
                            o_pss[ec],
                            ob_sb[:, bass.ts(ec, 512)],
                        )
                    xr = xrp.tile([P, 1024], f32)
                    nc.gpsimd.dma_start(
                        xr, xres[blk * 512 + tt * P : blk * 512 + (tt + 1) * P, :]
                    )
                    nc.vector.tensor_add(acc, acc, xr)
                    stats = stp.tile([P, 2, 6], f32, tag="stats")
                    nc.vector.bn_stats(stats[:, 0, :], acc[:, 0:512])
                    nc.vector.bn_stats(stats[:, 1, :], acc[:, 512:1024])
                    mv = stp.tile([P, 2], f32, tag="mv")
                    nc.vector.bn_aggr(mv, stats)
                    std = stp.tile([P, 1], f32, tag="std")
                    nc.scalar.activation(
                        std,
                        mv[:, 1:2],
                        mybir.ActivationFunctionType.Sqrt,
                        bias=eps_sb,
                        scale=1.0,
                    )
                    nc.vector.reciprocal(std, std)
                    nc.vector.tensor_scalar(
                        acc,
                        acc,
                        scalar1=mv[:, 0:1],
                        scalar2=std,
                        op0=mybir.AluOpType.subtract,
                        op1=mybir.AluOpType.mult,
                    )
                    nc.vector.tensor_mul(acc, acc, gam_sb)
                    nc.vector.tensor_add(acc, acc, bet_sb)
                    nc.sync.dma_start(
                        out[blk * 512 + tt * P : blk * 512 + (tt + 1) * P, :], acc
                    )
                coff += EXT
    return nc


# ----------------------------------------------------------------------
# Dispatch: L1 via run_bass_kernel_spmd (uniform); L2 via two concurrent
# shard_map programs on disjoint 4-device submeshes.
# ----------------------------------------------------------------------
_CACHE = {}
PROFILE = False
LAST_TIMING = {}


def _ensure_hook_shim():
    """Register antenv.axon_hooks (missing in this image) so that
    bass_utils' trace path and our own L2 profiling can drive NTFF
    capture through libaxon_pjrt.so."""
    import types
    import ctypes
    import contextlib

    try:
        from antenv.axon_hooks import get_axon_ntff_profile_hook  # noqa: F401
        return True
    except ImportError:
        pass
    so_path = "/opt/axon/libaxon_pjrt.so"
    if not os.path.exists(so_path):
        return False
    lib = ctypes.CDLL(so_path)
    if not hasattr(lib, "axon_start_nrt_profile"):
        return False
    lib.axon_start_nrt_profile.argtypes = [
        ctypes.POINTER(ctypes.c_int64),
        ctypes.c_size_t,
    ]
    lib.axon_start_nrt_profile.restype = ctypes.c_int64
    lib.axon_stop_nrt_profile.argtypes = [ctypes.c_char_p]
    lib.axon_stop_nrt_profile.restype = ctypes.c_int64

    @contextlib.contextmanager
    def _hook(output_dir, device_ids):
        import jax

        jax.devices()
        if device_ids:
            ids = (ctypes.c_int64 * len(device_ids))(*device_ids)
            rc = lib.axon_start_nrt_profile(ids, len(device_ids))
        else:
            rc = lib.axon_start_nrt_profile(None, 0)
        if rc != 0:
            raise RuntimeError(f"axon_start_nrt_profile rc={rc}")
        try:
            yield
        finally:
            n = lib.axon_stop_nrt_profile(str(output_dir).encode())
            if n < 0:
                raise RuntimeError(f"axon_stop_nrt_profile rc={n}")

    import antenv

    mod = types.ModuleType("antenv.axon_hooks")
    _state = {"hook": _hook}
    mod.get_axon_ntff_profile_hook = lambda: _state["hook"]
    mod.set_axon_ntff_profile_hook = lambda h: _state.__setitem__("hook", h)
    sys.modules["antenv.axon_hooks"] = mod
    antenv.axon_hooks = mod
    return True


def _process_l2_profile(nc, neff_dir, label, model_idx=0):
    import glob as _glob
    import gauge.profiler
    from concourse.bass_utils import _process_ntff_profile
    from concourse._compat import FishPath

    ntffs = _glob.glob(os.path.join(neff_dir, "*_body*.ntff"))
    if not ntffs:
        LAST_TIMING[label] = None
        return
    profile = gauge.profiler.Profile(
        profile_path=FishPath(neff_dir),
        kernel_dev_mode=True,
        profile_on_exit=False,
        bass_kernel=nc.m,
        offline_processing=True,
        fname="*_body*",
        metadata={},
    )
    perf = _process_ntff_profile(
        profile, neff_dir, nc, [model_idx], [model_idx], False, {"title": label}, False
    )
    LAST_TIMING[label] = perf.exec_time_ns
    LAST_TIMING[label + "_trace"] = perf.insts_and_trace_path


def _l2_exec(nc, n_cores):
    """Build a jitted shard_map executor for one L2 variant (modeled on
    bass2jax.run_bass_via_pjrt's multi-core branch, but device-list aware)."""
    import jax
    from jax.experimental.shard_map import shard_map
    from jax.sharding import Mesh, PartitionSpec
    from concourse import bass2jax

    bass2jax.install_neuronx_cc_hook()

    partition_name = nc.partition_id_tensor.name if nc.partition_id_tensor else None
    in_names, out_names, out_avals, zero_shapes = [], [], [], []
    for alloc in nc.m.functions[0].allocations:
        if not isinstance(alloc, mybir.MemoryLocationSet):
            continue
        name = alloc.memorylocations[0].name
        if alloc.kind == "ExternalInput":
            if name != partition_name:
                in_names.append(name)
        elif alloc.kind == "ExternalOutput":
            out_names.append(name)
            shape = tuple(alloc.tensor_shape)
            dtype = mybir.dt.np(alloc.dtype)
            out_avals.append(jax.core.ShapedArray(shape, dtype))
            zero_shapes.append((shape, dtype))
    n_params = len(in_names)
    all_names = in_names + out_names
    if partition_name is not None:
        all_names = all_names + [partition_name]
    donate = tuple(range(n_params, n_params + len(out_names)))

    def _body(*args):
        operands = list(args)
        if partition_name is not None:
            operands.append(bass2jax.partition_id_tensor())
        outs = bass2jax._bass_exec_p.bind(
            *operands,
            out_avals=tuple(out_avals),
            in_names=tuple(all_names),
            out_names=tuple(out_names),
            lowering_input_output_aliases=(),
            sim_require_finite=True,
            sim_require_nnan=True,
            nc=nc,
        )
        return tuple(outs)

    def run(in_maps, devices):
        assert len(in_maps) == n_cores
        mesh = Mesh(np.asarray(devices), ("core",))
        specs = (PartitionSpec("core"),) * (n_params + len(out_names))
        sharded = jax.jit(
            shard_map(
                _body,
                mesh=mesh,
                in_specs=specs,
                out_specs=(PartitionSpec("core"),) * len(out_names),
                check_rep=False,
            ),
            donate_argnums=donate,
            keep_unused=True,
        )
        concat_in = [
            np.concatenate([np.asarray(m[nm]) for m in in_maps], axis=0)
            for nm in in_names
        ]
        concat_zero = [
            np.zeros((n_cores * sh[0], *sh[1:]), dt) for sh, dt in zero_shapes
        ]
        return sharded(*concat_in, *concat_zero), out_names, out_avals

    return run


def _get_programs():
    if "l1" not in _CACHE:
        _CACHE["l1"] = build_l1()
        _CACHE["l2A"] = build_l2(VARIANT_A)
        _CACHE["l2B"] = build_l2(VARIANT_B)
        _CACHE["runA"] = _l2_exec(_CACHE["l2A"], 4)
        _CACHE["runB"] = _l2_exec(_CACHE["l2B"], 4)
    return _CACHE


def _block_rows(g):
    b, k = g // 4, g % 4
    return b, 512 * k, 512 * (k + 1)


def kernel(txt_emb, in_proj_w, in_proj_b, out_proj_w, out_proj_b, ln_gamma, ln_beta):
    import jax
    from concourse.bass_utils import run_bass_kernel_spmd

    progs = _get_programs()

    x = np.asarray(txt_emb, dtype=np.float32)
    wi = np.asarray(in_proj_w, dtype=np.float32)
    bi = np.asarray(in_proj_b, dtype=np.float32)
    wob = np.asarray(out_proj_w, dtype=np.float32)
    obb = np.asarray(out_proj_b, dtype=np.float32)
    gam = np.asarray(ln_gamma, dtype=np.float32)
    bet = np.asarray(ln_beta, dtype=np.float32)

    # ---- host prep (layouts) ----
    wT = np.ascontiguousarray(wi.T)  # [E, 3E]
    wqk_t = np.ascontiguousarray(
        wT[:, :2048].reshape(8, P, 16, P).transpose(2, 1, 0, 3)
    ).astype(BF16)  # [16, 128, 8, 128] = [ft, e_in, e_outer, f_in]
    wv_t = np.ascontiguousarray(wT[:, 2048:].reshape(8, P, 1024)).astype(BF16)
    bqk = np.ascontiguousarray(bi[:2048].reshape(16, P).T)  # [128, 16]
    bv = np.ascontiguousarray(bi[2048:])

    core_tokens = []  # per core [1024, 1024] natural x
    for c in range(8):
        rows = []
        for g in CORE_BLOCKS[c]:
            b, r0, r1 = _block_rows(g)
            rows.append(x[b, r0:r1, :])
        core_tokens.append(np.concatenate(rows, axis=0))

    l1_maps = []
    for c in range(8):
        xt = np.ascontiguousarray(core_tokens[c].T.reshape(8, P, 1024)).astype(BF16)
        l1_maps.append(
            {"xt": xt, "wqk": wqk_t, "wv": wv_t, "bqk": bqk, "bv": bv}
        )

    if PROFILE:
        _ensure_hook_shim()
        res1 = run_bass_kernel_spmd(
            progs["l1"], l1_maps, list(range(8)), trace=True
        )
        LAST_TIMING["l1"] = res1.exec_time_ns
        LAST_TIMING["l1_trace"] = res1.instructions_and_trace
    else:
        res1 = run_bass_kernel_spmd(progs["l1"], l1_maps, list(range(8)))

    # ---- assemble global K^T and V-aug ----
    KT = np.empty((B, 1024, 2048), dtype=BF16)
    VA = np.zeros((B, 2048, H, 72), dtype=BF16)
    for h in range(H):
        VA[:, :, h, 64 + h % 8] = np.asarray(1.0, dtype=BF16)
    QT = []  # per-core q^T [1024, 1024]
    for c in range(8):
        qkt = res1.results[c]["qkt"]
        vna = res1.results[c]["v"]
        QT.append(qkt[:1024])
        for slot, g in enumerate(CORE_BLOCKS[c]):
            b, r0, r1 = _block_rows(g)
            KT[b][:, r0:r1] = qkt[1024:, 512 * slot : 512 * (slot + 1)]
            VA[b, r0:r1, :, :64] = (
                vna[512 * slot : 512 * (slot + 1)].reshape(512, H, 64)
            )

    # ---- per-core L2 inputs ----
    mask = np.zeros((P, 896), dtype=BF16)
    yy = np.arange(896)[None, :]
    pp = np.arange(P)[:, None]
    mask[yy >= pp + 384] = np.asarray(1.0, dtype=BF16)
    onehot = np.zeros((P, 512), dtype=BF16)
    for hh in range(8):
        onehot[64 + hh, 64 * hh : 64 * hh + 64] = np.asarray(1.0, dtype=BF16)
    wo_t = np.ascontiguousarray(
        wob.T.reshape(8, P, 1024)
    ).astype(BF16)  # pair p rows: head 2p then head 2p+1

    l2_maps = {}
    for c in range(8):
        exts = _extents(c)
        kt_parts, va_parts = [], []
        for slot, g in enumerate(CORE_BLOCKS[c]):
            b, _, _ = _block_rows(g)
            kt_parts.append(KT[b][:, : exts[slot]])
            va_parts.append(VA[b][: exts[slot]])
        kt2 = np.ascontiguousarray(np.concatenate(kt_parts, axis=1))
        va = np.concatenate(va_parts, axis=0)  # [2560, 16, 72]
        va = np.ascontiguousarray(
            va.reshape(sum(exts), 8, 144).transpose(1, 0, 2)
        )
        qtz = np.zeros((2, 1024, 1024), dtype=BF16)
        qf = np.arange(1024)
        even = (qf // 64) % 2 == 0
        qtz[0, even] = QT[c][even]
        qtz[1, ~even] = QT[c][~even]
        l2_maps[c] = {
            "qt2": qtz,
            "kt2": kt2,
            "vaug": va,
            "xres": np.ascontiguousarray(core_tokens[c]),
            "wo": wo_t,
            "maskw": mask,
            "onehot": onehot,
            "ob": obb,
            "gamma": gam,
            "beta": bet,
        }

    devices = jax.devices()
    coresA = CORES_OF_VARIANT[VARIANT_A]
    coresB = CORES_OF_VARIANT[VARIANT_B]
    if PROFILE:
        import tempfile
        from antenv.axon_hooks import get_axon_ntff_profile_hook

        _ensure_hook_shim()
        from antenv.axon_hooks import get_axon_ntff_profile_hook

        hook = get_axon_ntff_profile_hook()
        dirA = tempfile.mkdtemp(prefix="l2A_")
        with hook(dirA, [devices[coresA[0]].id]):
            outA, namesA, avalsA = progs["runA"](
                [l2_maps[c] for c in coresA], [devices[c] for c in coresA]
            )
            jax.block_until_ready(outA)
        _process_l2_profile(progs["l2A"], dirA, "l2A", devices[coresA[0]].id)
        dirB = tempfile.mkdtemp(prefix="l2B_")
        with hook(dirB, [devices[coresB[0]].id]):
            outB, namesB, avalsB = progs["runB"](
                [l2_maps[c] for c in coresB], [devices[c] for c in coresB]
            )
            jax.block_until_ready(outB)
        _process_l2_profile(progs["l2B"], dirB, "l2B", devices[coresB[0]].id)
    else:
        outA, namesA, avalsA = progs["runA"](
            [l2_maps[c] for c in coresA], [devices[c] for c in coresA]
        )
        outB, namesB, avalsB = progs["runB"](
            [l2_maps[c] for c in coresB], [devices[c] for c in coresB]
        )

    result = np.empty((B, S, E), dtype=np.float32)

    def scatter(out_arrs, names, avals, cores):
        arr = np.asarray(out_arrs[names.index("out")]).reshape(
            len(cores), *avals[names.index("out")].shape
        )
        for ci, c in enumerate(cores):
            for slot, g in enumerate(CORE_BLOCKS[c]):
                b, r0, r1 = _block_rows(g)
                result[b, r0:r1, :] = arr[ci, 512 * slot : 512 * (slot + 1), :]

    scatter(outA, namesA, avalsA, coresA)
    scatter(outB, namesB, avalsB, coresB)
    return result
